# revision 1
# baseline (speedup 1.0000x reference)
"""ConvNeXt block (nn_CNBlock) Trainium2 Bass kernel.

Reference computation (per image, fp32):
  y = depthwise_conv7x7(x, conv_w) + conv_b          # NCHW, pad 3
  y = LayerNorm_channel(y) * ln_g + ln_b             # over C at each pixel
  h = gelu(y^T @ w1 + b1, exact)                     # C -> 4C
  out = h @ w2 + b2                                  # 4C -> C  (NCHW out)

Strategy: data-parallel over batch, 4 images per NeuronCore x 8 cores.
Per core, channels-first layout [C=2x128 partitions, pixels]:
  - conv: 32 taps on PE as diagonal-matrix matmuls (f32r) accumulated in
    PSUM + 17 taps as DVE fused scalar_tensor_tensor MACs (fp32).
  - LN: ones-matrix matmuls give per-pixel sums broadcast across all 128
    partitions in PSUM; variance/rsqrt via DVE/ACT; normalize on DVE.
    (ln affine folded into w1/b1 on host.)
  - MLP: f32r matmuls on PE, exact-erf Gelu + biases on ACT.
All matmul operands are float32r (TF32-like, ~1.5e-4 rel err, 4x faster
than fp32 on the PE).
"""
import sys

sys.path.insert(0, "/opt/trn_rl_repo")

import numpy as np

# ---------------- problem constants (hardcoded) ----------------
B, DIM, H, W = 32, 256, 56, 56
HID = 4 * DIM
EPS = 1e-6
NCORES = 8
BS = B // NCORES          # images per core
NCT = 2                   # channel tiles of 128
G = W + 6                 # padded grid width 62
NP = H * G                # conv output positions incl. garbage cols = 3472
XPL = 3856                # padded input tile length (3 + 62*62 + slack)
NCH = 8                   # pixel chunks
CW = NP // NCH            # chunk width 434 (= 7 rows of 62)
OW = 7 * W                # valid outputs per chunk 392
N_PE_TAPS = 32
N_DVE_TAPS = 49 - N_PE_TAPS

_CACHE = {}


def _taps():
    # (off, di, dj) for all 49 taps; off is the flat shift in the padded grid
    taps = []
    for di in range(7):
        for dj in range(7):
            taps.append((di * G + dj - 3, di, dj))
    return taps[:N_PE_TAPS], taps[N_PE_TAPS:]


def _build_program():
    import concourse.bacc as bacc
    import concourse.mybir as mybir
    import concourse.tile as tile

    dt = mybir.dt
    AF = mybir.ActivationFunctionType
    ALU = mybir.AluOpType
    F32R = dt.float32r
    F32 = dt.float32

    pe_taps, dve_taps = _taps()

    nc = bacc.Bacc("TRN2", target_bir_lowering=False, debug=False)

    d_xpad = nc.dram_tensor("xpad", [BS, NCT, 128, XPL], F32R, kind="ExternalInput")
    d_w1 = nc.dram_tensor("w1sb", [128, 2048], F32R, kind="ExternalInput")
    d_w2 = nc.dram_tensor("w2sb", [128, 2048], F32R, kind="ExternalInput")
    d_dg = nc.dram_tensor("dgsb", [128, NCT * N_PE_TAPS * 128], F32R, kind="ExternalInput")
    d_ones = nc.dram_tensor("ones128", [128, 128], F32R, kind="ExternalInput")
    # fp32 const columns: 0-1 cb, 2-9 b1eff, 10-11 b2, 12 eps, 13.. dve tap weights
    NC_CONST = 13 + NCT * N_DVE_TAPS
    d_cst = nc.dram_tensor("cstf", [128, NC_CONST], F32, kind="ExternalInput")
    d_out = nc.dram_tensor("yout", [BS, NCT, 128, H * W], F32, kind="ExternalOutput")

    with tile.TileContext(nc) as tc:
        with (
            tc.tile_pool(name="static", bufs=1) as stat,
            tc.tile_pool(name="xp", bufs=2) as p_xp,
            tc.tile_pool(name="yacc", bufs=3) as p_yacc,
            tc.tile_pool(name="y2", bufs=2) as p_y2,
            tc.tile_pool(name="yhat", bufs=2) as p_yhat,
            tc.tile_pool(name="hb", bufs=1) as p_h,
            tc.tile_pool(name="outc", bufs=2) as p_out,
            tc.tile_pool(name="var", bufs=1) as p_var,
            tc.tile_pool(name="ydve", bufs=1) as p_ydve,
            tc.tile_pool(name="pconv", bufs=2, space="PSUM") as ps_conv,
            tc.tile_pool(name="psy", bufs=1, space="PSUM") as ps_sy,
            tc.tile_pool(name="psy2", bufs=1, space="PSUM") as ps_sy2,
            tc.tile_pool(name="ph", bufs=2, space="PSUM") as ps_h,
            tc.tile_pool(name="po", bufs=2, space="PSUM") as ps_o,
        ):
            w1sb = stat.tile([128, 2048], F32R, name="w1sb")
            w2sb = stat.tile([128, 2048], F32R, name="w2sb")
            dgsb = stat.tile([128, NCT * N_PE_TAPS * 128], F32R, name="dgsb")
            ones128 = stat.tile([128, 128], F32R, name="ones128")
            cst = stat.tile([128, NC_CONST], F32, name="cst")
            HALF_DG = N_PE_TAPS * 128
            nc.sync.dma_start(dgsb[:, :HALF_DG], d_dg.ap()[:, :HALF_DG])
            nc.sync.dma_start(cst[:], d_cst.ap())
            nc.sync.dma_start(dgsb[:, HALF_DG:], d_dg.ap()[:, HALF_DG:])

            yaccs = {}
            vars_ = {}
            xps = {}

            def conv_dve(b):
                for ct in range(NCT):
                    xp = xps[(b, ct)]
                    ya = yaccs[(b, ct)]
                    yd = p_ydve.tile([128, NP], F32, name=f"yd_{b}_{ct}", tag="yd")
                    yd_v = yd[:, 0:NP].rearrange("p (r g) -> p r g", g=G)[:, :, 3:59]
                    for j, (off, _, _) in enumerate(dve_taps):
                        xv = xp[:, 6 + off: 6 + off + NP].rearrange(
                            "p (r g) -> p r g", g=G
                        )[:, :, 0:56]
                        wcol = cst[:, 13 + ct * N_DVE_TAPS + j: 14 + ct * N_DVE_TAPS + j]
                        if j == 0:
                            nc.vector.tensor_scalar(
                                yd_v, xv.bitcast(F32), wcol, None, op0=ALU.mult,
                            )
                        else:
                            nc.vector.scalar_tensor_tensor(
                                yd_v, xv.bitcast(F32), wcol, yd_v,
                                op0=ALU.mult, op1=ALU.add,
                            )
                    # merge DVE partial into ya on GpSimd (valid cols only),
                    # split in half to limit head-of-line blocking
                    ya_v = ya[:, 0:NP].rearrange("p (r g) -> p r g", g=G)[:, :, 3:59]
                    half = 28
                    nc.gpsimd.tensor_tensor(
                        ya_v[:, :half], ya_v[:, :half].bitcast(F32),
                        yd_v[:, :half], op=ALU.add)
                    nc.gpsimd.tensor_tensor(
                        ya_v[:, half:], ya_v[:, half:].bitcast(F32),
                        yd_v[:, half:], op=ALU.add)

            def conv_pe(b):
                for ct in range(NCT):
                    xp = p_xp.tile([128, XPL], F32R, name=f"xp_{b}_{ct}", tag="xp")
                    nc.sync.dma_start(xp[:], d_xpad.ap()[b, ct])
                    xps[(b, ct)] = xp
                    ya = p_yacc.tile([128, NP], F32R, name=f"ya_{b}_{ct}", tag="yacc")
                    yaccs[(b, ct)] = ya
                    for ch in range(NCH):
                        pc = ps_conv.tile([128, CW], F32, name=f"pc_{b}_{ct}_{ch}", tag="pc")
                        q0 = ch * CW
                        for i, (off, _, _) in enumerate(pe_taps):
                            nc.tensor.matmul(
                                pc[:],
                                dgsb[:, (ct * N_PE_TAPS + i) * 128:(ct * N_PE_TAPS + i + 1) * 128],
                                xp[:, 3 + off + q0: 3 + off + q0 + CW],
                                start=(i == 0),
                                stop=(i == N_PE_TAPS - 1),
                            )
                        nc.scalar.activation(
                            ya[:, q0:q0 + CW], pc[:], AF.Identity,
                            bias=cst[:, ct:ct + 1],
                        )

            def stats_phase(b):
                ya0 = yaccs[(b, 0)]
                ya1 = yaccs[(b, 1)]
                va = p_var.tile([128, NP], F32, name=f"va_{b}", tag="va")
                vars_[b] = va
                for ch in range(NCH):
                    q0 = ch * CW
                    sl = slice(q0, q0 + CW)
                    y2 = p_y2.tile([128, 2 * CW], F32R, name=f"y2_{b}_{ch}", tag="y2")
                    nc.scalar.activation(y2[:, 0:CW], ya0[:, sl].bitcast(F32), AF.Square)
                    nc.scalar.activation(y2[:, CW:], ya1[:, sl].bitcast(F32), AF.Square)
                    psy = ps_sy.tile([128, CW], F32, name=f"psy_{b}_{ch}", tag="psy")
                    nc.tensor.matmul(psy[:], ones128[:], ya0[:, sl], start=True, stop=False)
                    nc.tensor.matmul(psy[:], ones128[:], ya1[:, sl], start=False, stop=True)
                    psy2 = ps_sy2.tile([128, CW], F32, name=f"psy2_{b}_{ch}", tag="psy2")
                    nc.tensor.matmul(psy2[:], ones128[:], y2[:, 0:CW], start=True, stop=False)
                    nc.tensor.matmul(psy2[:], ones128[:], y2[:, CW:], start=False, stop=True)
                    # var slice: mu, mu^2, then sy2/256 - mu^2
                    vsl = va[:, sl]
                    nc.scalar.activation(vsl, psy[:], AF.Copy, bias=0.0, scale=1.0 / DIM)
                    nc.scalar.activation(vsl, vsl, AF.Square)
                    nc.vector.scalar_tensor_tensor(
                        vsl, psy2[:], 1.0 / DIM, vsl, op0=ALU.mult, op1=ALU.subtract
                    )
                    # center y in place: y -= mu
                    for ya in (ya0, ya1):
                        nc.vector.scalar_tensor_tensor(
                            ya[:, sl], psy[:], -1.0 / DIM, ya[:, sl].bitcast(F32),
                            op0=ALU.mult, op1=ALU.add,
                        )
                # batched rsqrt: r = 1/sqrt(var + eps), in place, one table switch
                nc.scalar.activation(va[:], va[:], AF.Abs_reciprocal_sqrt, bias=cst[:, 12:13])

            def mlp_phase(b):
                ya0 = yaccs[(b, 0)]
                ya1 = yaccs[(b, 1)]
                va = vars_[b]
                for ch in range(NCH):
                    q0 = ch * CW
                    sl = slice(q0, q0 + CW)
                    yh = p_yhat.tile([128, 2 * CW], F32R, name=f"yh_{b}_{ch}", tag="yh")
                    for ct, ya in ((0, ya0), (1, ya1)):
                        nc.gpsimd.tensor_tensor(
                            yh[:, ct * CW:(ct + 1) * CW], ya[:, sl].bitcast(F32),
                            va[:, sl], op=ALU.mult,
                        )
                    hb = p_h.tile([128, 8 * CW], F32R, name=f"hb_{b}_{ch}", tag="hb")
                    for f in range(8):
                        ph = ps_h.tile([128, CW], F32, name=f"ph_{b}_{ch}_{f}", tag="ph")
                        nc.tensor.matmul(
                            ph[:], w1sb[:, f * 128:(f + 1) * 128], yh[:, 0:CW],
                            start=True, stop=False,
                        )
                        nc.tensor.matmul(
                            ph[:], w1sb[:, 1024 + f * 128:1024 + (f + 1) * 128],
                            yh[:, CW:], start=False, stop=True,
                        )
                        nc.scalar.activation(
                            hb[:, f * CW:(f + 1) * CW], ph[:], AF.Gelu,
                            bias=cst[:, 2 + f:3 + f],
                        )
                    oc = p_out.tile([128, 2 * CW], F32, name=f"oc_{b}_{ch}", tag="oc")
                    for ct in range(NCT):
                        po = ps_o.tile([128, CW], F32, name=f"po_{b}_{ch}_{ct}", tag="po")
                        for f in range(8):
                            nc.tensor.matmul(
                                po[:], w2sb[:, f * 256 + ct * 128: f * 256 + (ct + 1) * 128],
                                hb[:, f * CW:(f + 1) * CW],
                                start=(f == 0), stop=(f == 7),
                            )
                        nc.scalar.activation(
                            oc[:, ct * CW:(ct + 1) * CW], po[:], AF.Identity,
                            bias=cst[:, 10 + ct:11 + ct],
                        )
                        src = oc[:, ct * CW:(ct + 1) * CW].rearrange(
                            "p (r w) -> p r w", r=7
                        )[:, :, 3:59]
                        dst = d_out.ap()[b, ct, :, ch * OW:(ch + 1) * OW].rearrange(
                            "p (r w) -> p r w", w=W
                        )
                        nc.sync.dma_start(dst, src)

            # software pipeline: conv one image ahead; stats DVE ops are
            # emitted before the next image's DVE tap chains
            conv_pe(0); conv_dve(0)
            nc.sync.dma_start(w1sb[:], d_w1.ap())
            nc.sync.dma_start(w2sb[:], d_w2.ap())
            nc.sync.dma_start(ones128[:], d_ones.ap())
            conv_pe(1)
            stats_phase(0)
            conv_dve(1)
            for b in range(BS):
                mlp_phase(b)
                if b + 1 < BS:
                    stats_phase(b + 1)
                if b + 2 < BS:
                    conv_pe(b + 2)
                    conv_dve(b + 2)

    nc.compile()
    return nc


def _host_prep(x, conv_w, conv_b, ln_g, ln_b, w1, b1, w2, b2):
    """Returns (shared static arrays dict, per-core xpad list)."""
    f32 = np.float32
    x = np.asarray(x, f32)
    conv_w = np.asarray(conv_w, f32)
    conv_b = np.asarray(conv_b, f32)
    ln_g = np.asarray(ln_g, f32)
    ln_b = np.asarray(ln_b, f32)
    w1 = np.asarray(w1, f32)
    b1 = np.asarray(b1, f32)
    w2 = np.asarray(w2, f32)
    b2 = np.asarray(b2, f32)

    pe_taps, dve_taps = _taps()

    # fold LN affine into w1/b1
    w1g = (ln_g[:, None] * w1).astype(f32)                  # [256, 1024]
    b1e = (ln_b @ w1 + b1).astype(f32)                      # [1024]

    # w1sb[c, ct*1024 + f*128 + j] = w1g[ct*128 + c, f*128 + j]
    w1sb = np.ascontiguousarray(
        w1g.reshape(2, 128, 8, 128).transpose(1, 0, 2, 3).reshape(128, 2048)
    )
    # w2sb[h, f*256 + ct*128 + co] = w2[f*128 + h, ct*128 + co]
    w2sb = np.ascontiguousarray(
        w2.reshape(8, 128, 2, 128).transpose(1, 0, 2, 3).reshape(128, 2048)
    )
    # diagonal conv matrices for PE taps
    dgsb = np.zeros((128, NCT * N_PE_TAPS * 128), f32)
    idx = np.arange(128)
    for ct in range(NCT):
        for i, (_, di, dj) in enumerate(pe_taps):
            dgsb[idx, (ct * N_PE_TAPS + i) * 128 + idx] = conv_w[ct * 128 + idx, 0, di, dj]
    ones128 = np.ones((128, 128), f32)

    NC_CONST = 13 + NCT * N_DVE_TAPS
    cst = np.zeros((128, NC_CONST), f32)
    cst[:, 0] = conv_b[:128]
    cst[:, 1] = conv_b[128:]
    cst[:, 2:10] = b1e.reshape(8, 128).T
    cst[:, 10] = b2[:128]
    cst[:, 11] = b2[128:]
    cst[:, 12] = EPS
    for ct in range(NCT):
        for j, (_, di, dj) in enumerate(dve_taps):
            cst[:, 13 + ct * N_DVE_TAPS + j] = conv_w[ct * 128 + idx, 0, di, dj]

    # padded input grids
    xg = np.zeros((B, DIM, G, G), f32)
    xg[:, :, 3:59, 3:59] = x
    xg = xg.reshape(B, NCT, 128, G * G)
    xpad = np.zeros((B, NCT, 128, XPL), f32)
    xpad[:, :, :, 3:3 + G * G] = xg

    static = dict(w1sb=w1sb, w2sb=w2sb, dgsb=dgsb, ones128=ones128, cstf=cst)
    xpads = [np.ascontiguousarray(xpad[c * BS:(c + 1) * BS]) for c in range(NCORES)]
    return static, xpads


def kernel(**inputs) -> np.ndarray:
    from concourse import bass_utils

    if "nc" not in _CACHE:
        _CACHE["nc"] = _build_program()
    nc = _CACHE["nc"]

    static, xpads = _host_prep(**inputs)
    in_maps = [dict(static, xpad=xpads[c]) for c in range(NCORES)]
    res = bass_utils.run_bass_kernel_spmd(nc, in_maps, core_ids=list(range(NCORES)))

    out = np.empty((B, DIM, H, W), np.float32)
    for c in range(NCORES):
        yo = res.results[c]["yout"].reshape(BS, NCT, 128, H, W)
        for b in range(BS):
            out[c * BS + b, :128] = yo[b, 0]
            out[c * BS + b, 128:] = yo[b, 1]
    return out



# revision 2
# speedup vs baseline: 1.0205x; 1.0205x over previous
"""ConvNeXt block (nn_CNBlock) Trainium2 Bass kernel — v2 (banded conv).

Reference computation (per image, fp32):
  y = depthwise_conv7x7(x, conv_w) + conv_b          # NCHW, pad 3
  y = LayerNorm_channel(y) * ln_g + ln_b             # over C at each pixel
  h = gelu(y^T @ w1 + b1, exact)                     # C -> 4C
  out = h @ w2 + b2                                  # 4C -> C  (NCHW out)

Strategy: data-parallel over batch, 4 images per core x 8 cores.

Conv as banded-Toeplitz matmuls on the PE: partitions hold (2 channels x
62 padded rows); the stationary for (channel-pair q, dj) is a banded
matrix encoding all 7 vertical taps, so each pair needs only 7 matmuls
of 224 columns (4 images x 56 output cols) with PSUM accumulation over
dj.  Conv bias rides in as an extra K row (moving operand row == 1.0).
Conv output is row-major, so it is transposed back to channel-major via
a DRAM bounce: scatter-writes (per 16-pair group) then 4 big contiguous
reads.  LN stats (ones-matmul trick) + MLP identical in structure to the
diag-conv kernel, but with bf16 activations/weights (PSUM accumulation
stays fp32) and 448-column chunks (8 image rows).
"""
import sys

sys.path.insert(0, "/opt/trn_rl_repo")

import numpy as np

# ---------------- problem constants (hardcoded) ----------------
B, DIM, H, W = 32, 256, 56, 56
HID = 4 * DIM
EPS = 1e-6
NCORES = 8
BS = B // NCORES          # images per core
NCT = 2                   # channel tiles of 128
NQ = 64                   # channel pairs per ct
GR = 62                   # padded rows
GJ = 62                   # padded cols
PIX = H * W               # 3136
PIXT = BS * PIX           # 12544
CW = 448                  # mlp/stats chunk: 8 image rows
NRB = PIX // CW           # 7 chunks per image
CCOL = BS * W             # conv matmul moving cols = 224

_CACHE = {}


def _build_program():
    import concourse.bacc as bacc
    import concourse.mybir as mybir
    import concourse.tile as tile

    dt = mybir.dt
    AF = mybir.ActivationFunctionType
    ALU = mybir.AluOpType
    BF16 = dt.bfloat16
    F32 = dt.float32
    I32 = dt.int32
    MAGIC = 0x5F3759DF  # rsqrt bit-trick seed constant

    nc = bacc.Bacc("TRN2", target_bir_lowering=False, debug=False)

    # pre-swizzled on host: per-partition-contiguous group tiles
    d_xcv = nc.dram_tensor("xcv", [8, 128, 16 * BS * GJ], BF16, kind="ExternalInput")
    d_wsb = nc.dram_tensor("wsb", [64, 128, 2 * 7 * 128], BF16, kind="ExternalInput")
    d_w1 = nc.dram_tensor("w1sb", [128, 2048], BF16, kind="ExternalInput")
    d_w2 = nc.dram_tensor("w2sb", [128, 2048], BF16, kind="ExternalInput")
    d_ones = nc.dram_tensor("ones128", [128, 128], BF16, kind="ExternalInput")
    # fp32 const columns: 0-7 b1eff, 8-9 b2, 10 eps
    d_cst = nc.dram_tensor("cstf", [128, 11], F32, kind="ExternalInput")
    d_out = nc.dram_tensor("yout", [BS, NCT, 128, PIX], F32, kind="ExternalOutput")
    # transpose bounce buffer: [ct, c2, b, q, r, j] (scatter on the write side)
    d_T = nc.dram_tensor("tbounce", [NCT, 2, BS, NQ, H, W], BF16, kind="Internal")

    with tile.TileContext(nc) as tc:
        with (
            tc.tile_pool(name="static", bufs=1) as stat,
            tc.tile_pool(name="xg", bufs=3) as p_xg,
            tc.tile_pool(name="wst", bufs=6) as p_wst,
            tc.tile_pool(name="s1", bufs=2) as p_s1,
            tc.tile_pool(name="ybuf", bufs=1) as p_y,
            tc.tile_pool(name="y2", bufs=2) as p_y2,
            tc.tile_pool(name="yh", bufs=2) as p_yh,
            tc.tile_pool(name="hb", bufs=2) as p_h,
            tc.tile_pool(name="oc", bufs=3) as p_oc,
            tc.tile_pool(name="mu", bufs=2) as p_mu,
            tc.tile_pool(name="va", bufs=1) as p_va,
            tc.tile_pool(name="vr", bufs=2) as p_vr,
            tc.tile_pool(name="pconv", bufs=2, space="PSUM") as ps_conv,
            tc.tile_pool(name="psy", bufs=1, space="PSUM") as ps_sy,
            tc.tile_pool(name="psy2", bufs=1, space="PSUM") as ps_sy2,
            tc.tile_pool(name="ph", bufs=2, space="PSUM") as ps_h,
            tc.tile_pool(name="po", bufs=2, space="PSUM") as ps_o,
        ):
            w1sb = stat.tile([128, 2048], BF16, name="w1sb")
            w2sb = stat.tile([128, 2048], BF16, name="w2sb")
            ones128 = stat.tile([128, 128], BF16, name="ones128")
            cst = stat.tile([128, 11], F32, name="cst")
            nc.scalar.dma_start(cst[:], d_cst.ap())
            nc.scalar.dma_start(ones128[:], d_ones.ap())
            nc.scalar.dma_start(w1sb[:], d_w1.ap())
            nc.scalar.dma_start(w2sb[:], d_w2.ap())

            Y = [
                stat.tile([128, PIXT], BF16, name="Y0"),
                stat.tile([128, PIXT], BF16, name="Y1"),
            ]

            # ---------------- phase A: conv + transpose ----------------
            # super-group of 32 channel pairs: fewer, larger T writes
            def conv_sgroup(ct, G):
                s1 = p_s1.tile([128, 32 * CCOL], BF16, name=f"s1_{ct}_{G}", tag="s1")
                for half in range(2):
                    g = G * 2 + half
                    t0 = ct * NQ + g * 16
                    xg = p_xg.tile(
                        [128, 16 * BS * GJ], BF16, name=f"xg_{ct}_{g}", tag="xg"
                    )
                    nc.sync.dma_start(xg[:], d_xcv.ap()[ct * 4 + g])
                    xgv = xg[:].rearrange("p (q b jj) -> p q b jj", q=16, b=BS)
                    for qi in range(16):
                        if qi % 2 == 0:
                            wst = p_wst.tile(
                                [128, 2 * 7 * 128], BF16,
                                name=f"ws_{ct}_{g}_{qi}", tag="wst",
                            )
                            nc.sync.dma_start(wst[:], d_wsb.ap()[(t0 + qi) // 2])
                        pc = ps_conv.tile(
                            [128, CCOL], F32, name=f"pc_{ct}_{g}_{qi}", tag="pc"
                        )
                        wbase = (qi % 2) * 896
                        for dj in range(7):
                            nc.tensor.matmul(
                                pc[:],
                                wst[:, wbase + dj * 128:wbase + (dj + 1) * 128],
                                xgv[:, qi, :, dj:dj + W],
                                start=(dj == 0),
                                stop=(dj == 6),
                            )
                        qo = half * 16 + qi
                        nc.vector.tensor_copy(s1[:, qo * CCOL:(qo + 1) * CCOL], pc[:])
                s1v = s1[:].rearrange("p (q b j) -> p q b j", q=32, b=BS)
                # scatter-writes on the 2nd HWDGE ring (ACT is idle in
                # phase A) so they don't block the weight stream
                for c2 in range(2):
                    for b in range(BS):
                        nc.scalar.dma_start(
                            d_T.ap()[ct, c2, b, G * 32:(G + 1) * 32].rearrange(
                                "q r j -> r q j"
                            ),
                            s1v[c2 * 64:c2 * 64 + H, :, b, :],
                        )

            def read_y(ct, G):
                # Y partition layout: c = 2q + c2, so each 64-partition read
                # depends only on super-group G's T writes (tiny launch wait)
                for b in range(BS):
                    nc.sync.dma_start(
                        Y[ct][G * 64:(G + 1) * 64, b * PIX:(b + 1) * PIX],
                        d_T.ap()[ct, :, b, G * 32:(G + 1) * 32].rearrange(
                            "c2 q r j -> q c2 (r j)"
                        ),
                    )

            # emit each Y read one super-group late: its T-write deps are then
            # already (nearly) complete, so the launch never blocks the ring
            sgs = [(0, 0), (0, 1), (1, 0), (1, 1)]
            for k, (ct, G) in enumerate(sgs):
                conv_sgroup(ct, G)
                if k >= 1:
                    read_y(*sgs[k - 1])
            read_y(*sgs[-1])

            # ---------------- phase B: LN stats + MLP ----------------
            mus = {}
            vrs = {}

            def stats_chunk(i, rb):
                sl = slice((i * NRB + rb) * CW, (i * NRB + rb + 1) * CW)
                msl = slice(rb * CW, (rb + 1) * CW)
                if rb == 0:
                    mus[i] = p_mu.tile([128, PIX], F32, name=f"mu_{i}", tag="mu")
                    mus["va", i] = p_va.tile([128, PIX], F32, name=f"va_{i}", tag="va")
                mu = mus[i]
                va = mus["va", i]
                y2 = p_y2.tile([128, 2 * CW], BF16, name=f"y2_{i}_{rb}", tag="y2")
                nc.vector.tensor_tensor(y2[:, 0:CW], Y[0][:, sl], Y[0][:, sl], op=ALU.mult)
                nc.vector.tensor_tensor(y2[:, CW:], Y[1][:, sl], Y[1][:, sl], op=ALU.mult)
                psy = ps_sy.tile([128, CW], F32, name=f"psy_{i}_{rb}", tag="psy")
                nc.tensor.matmul(psy[:], ones128[:], Y[0][:, sl], start=True, stop=False)
                nc.tensor.matmul(psy[:], ones128[:], Y[1][:, sl], start=False, stop=True)
                psy2 = ps_sy2.tile([128, CW], F32, name=f"psy2_{i}_{rb}", tag="psy2")
                nc.tensor.matmul(psy2[:], ones128[:], y2[:, 0:CW], start=True, stop=False)
                nc.tensor.matmul(psy2[:], ones128[:], y2[:, CW:], start=False, stop=True)
                nc.vector.tensor_scalar(mu[:, msl], psy[:], 1.0 / DIM, None, op0=ALU.mult)
                nc.vector.tensor_tensor(va[:, msl], mu[:, msl], mu[:, msl], op=ALU.mult)
                nc.vector.scalar_tensor_tensor(
                    va[:, msl], psy2[:], 1.0 / DIM, va[:, msl],
                    op0=ALU.mult, op1=ALU.subtract,
                )
            def stats_fin(i):
                # r = 1/sqrt(var + eps), batched per image (one ACT table use)
                vr = p_vr.tile([128, PIX], BF16, name=f"vr_{i}", tag="vr")
                vrs[i] = vr
                nc.scalar.activation(
                    vr[:], mus["va", i][:], AF.Abs_reciprocal_sqrt, bias=cst[:, 10:11]
                )

            def mlp_chunk(i, rb):
                sl = slice((i * NRB + rb) * CW, (i * NRB + rb + 1) * CW)
                msl = slice(rb * CW, (rb + 1) * CW)
                mu = mus[i]
                vr = vrs[i]
                yh = p_yh.tile([128, 2 * CW], BF16, name=f"yh_{i}_{rb}", tag="yh")
                for ct2 in range(2):
                    ysl = yh[:, ct2 * CW:(ct2 + 1) * CW]
                    nc.vector.scalar_tensor_tensor(
                        ysl, mu[:, msl], -1.0, Y[ct2][:, sl], op0=ALU.mult, op1=ALU.add
                    )
                    nc.gpsimd.tensor_tensor(ysl, ysl, vr[:, msl], op=ALU.mult)
                hb = p_h.tile([128, 8 * CW], BF16, name=f"hb_{i}_{rb}", tag="hb")
                for f in range(8):
                    ph = ps_h.tile([128, CW], F32, name=f"ph_{i}_{rb}_{f}", tag="ph")
                    nc.tensor.matmul(
                        ph[:], w1sb[:, f * 128:(f + 1) * 128], yh[:, 0:CW],
                        start=True, stop=False,
                    )
                    nc.tensor.matmul(
                        ph[:], w1sb[:, 1024 + f * 128:1024 + (f + 1) * 128],
                        yh[:, CW:], start=False, stop=True,
                    )
                    nc.scalar.activation(
                        hb[:, f * CW:(f + 1) * CW], ph[:], AF.Gelu,
                        bias=cst[:, f:f + 1],
                    )
                for ct2 in range(2):
                    po = ps_o.tile([128, CW], F32, name=f"po_{i}_{rb}_{ct2}", tag="po")
                    for f in range(8):
                        nc.tensor.matmul(
                            po[:], w2sb[:, f * 256 + ct2 * 128:f * 256 + (ct2 + 1) * 128],
                            hb[:, f * CW:(f + 1) * CW],
                            start=(f == 0), stop=(f == 7),
                        )
                    oc = p_oc.tile([128, CW], F32, name=f"oc_{i}_{rb}_{ct2}", tag="oc")
                    nc.vector.tensor_scalar(
                        oc[:], po[:], cst[:, 8 + ct2:9 + ct2], None, op0=ALU.add
                    )
                    nc.sync.dma_start(
                        d_out.ap()[i, ct2, :, rb * CW:(rb + 1) * CW], oc[:]
                    )

            def phase_b():
                for rb in range(NRB):
                    stats_chunk(0, rb)
                stats_fin(0)
                # front-load image i+1's stats into the first half of image
                # i's MLP so the rsqrt hides under the remaining MLP matmuls
                stats_slot = {0: [0, 1], 1: [2, 3], 2: [4, 5], 3: [6]}
                for i in range(BS):
                    for rb in range(NRB):
                        mlp_chunk(i, rb)
                        if i + 1 < BS:
                            for sc in stats_slot.get(rb, []):
                                stats_chunk(i + 1, sc)
                            if rb == 3:
                                stats_fin(i + 1)

            phase_b()

    nc.compile()
    return nc


def _host_prep(x, conv_w, conv_b, ln_g, ln_b, w1, b1, w2, b2):
    """Returns (shared static arrays dict, per-core xcv list)."""
    import concourse.mybir as mybir

    f32 = np.float32
    bf16 = np.dtype(mybir.dt.np(mybir.dt.bfloat16))
    x = np.asarray(x, f32)
    conv_w = np.asarray(conv_w, f32)
    conv_b = np.asarray(conv_b, f32)
    ln_g = np.asarray(ln_g, f32)
    ln_b = np.asarray(ln_b, f32)
    w1 = np.asarray(w1, f32)
    b1 = np.asarray(b1, f32)
    w2 = np.asarray(w2, f32)
    b2 = np.asarray(b2, f32)

    # fold LN affine into w1/b1
    w1g = (ln_g[:, None] * w1).astype(f32)                  # [256, 1024]
    b1e = (ln_b @ w1 + b1).astype(f32)                      # [1024]

    # Y partition layout is c = 2q + c2 (so per-super-group reads are
    # contiguous); permute w1's contraction rows to match
    qv = np.arange(NQ)
    perm = np.empty(128, np.int64)
    perm[2 * qv] = qv
    perm[2 * qv + 1] = 64 + qv
    w1sb = np.ascontiguousarray(
        w1g.reshape(2, 128, 8, 128)[:, perm].transpose(1, 0, 2, 3).reshape(128, 2048)
    ).astype(bf16)
    w2sb = np.ascontiguousarray(
        w2.reshape(8, 128, 2, 128).transpose(1, 0, 2, 3).reshape(128, 2048)
    ).astype(bf16)
    ones128 = np.ones((128, 128), bf16)

    cst = np.zeros((128, 11), f32)
    cst[:, 0:8] = b1e.reshape(8, 128).T
    cst[:, 8] = b2[:128]
    cst[:, 9] = b2[128:]
    cst[:, 10] = EPS

    # banded stationaries: wsb[ct, q, k, dj, m]
    wsb = np.zeros((2, NQ, 128, 7, 128), f32)
    w4 = conv_w.reshape(2, 2, NQ, 7, 7)                     # [ct, c2, q, di, dj]
    cb4 = conv_b.reshape(2, 2, NQ)
    ro = np.arange(H)
    for c2 in range(2):
        for di in range(7):
            for dj in range(7):
                wsb[:, :, c2 * 64 + ro + di, dj, c2 * 64 + ro] = \
                    w4[:, c2, :, di, dj][:, :, None]
        wsb[:, :, 62, 0, c2 * 64:c2 * 64 + H] = cb4[:, c2, :, None]
    # regroup: pairs of 2 q-tiles, per-partition contiguous
    wsb = np.ascontiguousarray(
        wsb.reshape(64, 2, 128, 7 * 128).transpose(0, 2, 1, 3).reshape(64, 128, 1792)
    ).astype(bf16)

    # padded input grids, conv layout
    xg = np.zeros((B, DIM, GR, GJ), f32)
    xg[:, :, 3:59, 3:59] = x
    xcvs = []
    for c in range(NCORES):
        xc = xg[c * BS:(c + 1) * BS].reshape(BS, 2, 2, NQ, GR, GJ)
        xcv = np.zeros((2, NQ, 128, BS, GJ), f32)
        xct = xc.transpose(1, 3, 2, 4, 0, 5)                # [ct, q, c2, r, b, jj]
        for c2 in range(2):
            xcv[:, :, c2 * 64:c2 * 64 + GR] = xct[:, :, c2]
        xcv[:, :, 62] = 1.0
        xcvs.append(
            np.ascontiguousarray(
                xcv.reshape(8, 16, 128, BS * GJ)
                .transpose(0, 2, 1, 3)
                .reshape(8, 128, 16 * BS * GJ)
            ).astype(bf16)
        )

    static = dict(w1sb=w1sb, w2sb=w2sb, wsb=wsb, ones128=ones128, cstf=cst)
    return static, xcvs


def kernel(**inputs) -> np.ndarray:
    from concourse import bass_utils

    if "nc" not in _CACHE:
        _CACHE["nc"] = _build_program()
    nc = _CACHE["nc"]

    static, xcvs = _host_prep(**inputs)
    in_maps = [dict(static, xcv=xcvs[c]) for c in range(NCORES)]
    res = bass_utils.run_bass_kernel_spmd(nc, in_maps, core_ids=list(range(NCORES)))

    out = np.empty((B, DIM, H, W), np.float32)
    for c in range(NCORES):
        yo = res.results[c]["yout"].reshape(BS, NCT, 128, H, W)
        for b in range(BS):
            out[c * BS + b, :128] = yo[b, 0]
            out[c * BS + b, 128:] = yo[b, 1]
    return out


# revision 4
# speedup vs baseline: 1.0419x; 1.0209x over previous
"""ConvNeXt block (nn_CNBlock) Trainium2 Bass kernel — v2 (banded conv).

Reference computation (per image, fp32):
  y = depthwise_conv7x7(x, conv_w) + conv_b          # NCHW, pad 3
  y = LayerNorm_channel(y) * ln_g + ln_b             # over C at each pixel
  h = gelu(y^T @ w1 + b1, exact)                     # C -> 4C
  out = h @ w2 + b2                                  # 4C -> C  (NCHW out)

Strategy: data-parallel over batch, 4 images per core x 8 cores.

Conv as banded-Toeplitz matmuls on the PE: partitions hold (2 channels x
62 padded rows); the stationary for (channel-pair q, dj) is a banded
matrix encoding all 7 vertical taps, so each pair needs only 7 matmuls
of 224 columns (4 images x 56 output cols) with PSUM accumulation over
dj.  Conv bias rides in as an extra K row (moving operand row == 1.0).
Conv output is row-major, so it is transposed back to channel-major via
a DRAM bounce: scatter-writes (per 16-pair group) then 4 big contiguous
reads.  LN stats (ones-matmul trick) + MLP identical in structure to the
diag-conv kernel, but with bf16 activations/weights (PSUM accumulation
stays fp32) and 448-column chunks (8 image rows).
"""
import sys

sys.path.insert(0, "/opt/trn_rl_repo")

import numpy as np

# ---------------- problem constants (hardcoded) ----------------
B, DIM, H, W = 32, 256, 56, 56
HID = 4 * DIM
EPS = 1e-6
NCORES = 8
BS = B // NCORES          # images per core
NCT = 2                   # channel tiles of 128
NQ = 64                   # channel pairs per ct
GR = 62                   # padded rows
GJ = 62                   # padded cols
PIX = H * W               # 3136
PIXT = BS * PIX           # 12544
CW = 448                  # mlp/stats chunk: 8 image rows
NRB = PIX // CW           # 7 chunks per image
CCOL = BS * W             # conv matmul moving cols = 224

_CACHE = {}


def _build_program():
    import concourse.bacc as bacc
    import concourse.mybir as mybir
    import concourse.tile as tile

    dt = mybir.dt
    AF = mybir.ActivationFunctionType
    ALU = mybir.AluOpType
    BF16 = dt.bfloat16
    F32 = dt.float32
    I32 = dt.int32
    MAGIC = 0x5F3759DF  # rsqrt bit-trick seed constant

    nc = bacc.Bacc("TRN2", target_bir_lowering=False, debug=False)

    # pre-swizzled on host: per-partition-contiguous group tiles
    d_xcv = nc.dram_tensor("xcv", [8, 128, 16 * BS * GJ], BF16, kind="ExternalInput")
    d_wsb = nc.dram_tensor("wsb", [64, 128, 2 * 7 * 128], BF16, kind="ExternalInput")
    d_w1 = nc.dram_tensor("w1sb", [128, 2048], BF16, kind="ExternalInput")
    d_w2 = nc.dram_tensor("w2sb", [128, 2048], BF16, kind="ExternalInput")
    d_ones = nc.dram_tensor("ones128", [128, 128], BF16, kind="ExternalInput")
    # fp32 const columns: 0-7 b1eff, 8-9 b2, 10 eps
    d_cst = nc.dram_tensor("cstf", [128, 11], F32, kind="ExternalInput")
    d_out = nc.dram_tensor("yout", [BS, NCT, 128, PIX], F32, kind="ExternalOutput")
    # transpose bounce buffer: [ct, c2, b, q, r, j] (scatter on the write side)
    d_T = nc.dram_tensor("tbounce", [NCT, 2, BS, NQ, H, W], BF16, kind="Internal")

    with tile.TileContext(nc) as tc:
        with (
            tc.tile_pool(name="static", bufs=1) as stat,
            tc.tile_pool(name="xg", bufs=3) as p_xg,
            tc.tile_pool(name="wst", bufs=6) as p_wst,
            tc.tile_pool(name="s1", bufs=2) as p_s1,
            tc.tile_pool(name="ybuf", bufs=1) as p_y,
            tc.tile_pool(name="y2", bufs=2) as p_y2,
            tc.tile_pool(name="yh", bufs=2) as p_yh,
            tc.tile_pool(name="hb", bufs=2) as p_h,
            tc.tile_pool(name="oc", bufs=3) as p_oc,
            tc.tile_pool(name="mu", bufs=2) as p_mu,
            tc.tile_pool(name="va", bufs=1) as p_va,
            tc.tile_pool(name="vr", bufs=2) as p_vr,
            tc.tile_pool(name="pconv", bufs=2, space="PSUM") as ps_conv,
            tc.tile_pool(name="psy", bufs=1, space="PSUM") as ps_sy,
            tc.tile_pool(name="psy2", bufs=1, space="PSUM") as ps_sy2,
            tc.tile_pool(name="ph", bufs=2, space="PSUM") as ps_h,
            tc.tile_pool(name="po", bufs=2, space="PSUM") as ps_o,
        ):
            w1sb = stat.tile([128, 2048], BF16, name="w1sb")
            w2sb = stat.tile([128, 2048], BF16, name="w2sb")
            ones128 = stat.tile([128, 128], BF16, name="ones128")
            cst = stat.tile([128, 11], F32, name="cst")
            nc.scalar.dma_start(cst[:], d_cst.ap())
            nc.scalar.dma_start(ones128[:], d_ones.ap())
            nc.scalar.dma_start(w1sb[:], d_w1.ap())
            nc.scalar.dma_start(w2sb[:], d_w2.ap())

            Y = [
                stat.tile([128, PIXT], BF16, name="Y0"),
                stat.tile([128, PIXT], BF16, name="Y1"),
            ]

            # ---------------- phase A: conv + transpose ----------------
            xgs = {}

            def ensure_xg(gk):
                # prefetch the conv input tile one half-group ahead so the
                # stream never arrives just-in-time-late at group boundaries
                if gk in xgs or gk >= 8:
                    return
                xg = p_xg.tile([128, 16 * BS * GJ], BF16, name=f"xg_{gk}", tag="xg")
                nc.sync.dma_start(xg[:], d_xcv.ap()[gk])
                xgs[gk] = xg

            # super-group of 32 channel pairs: fewer, larger T writes
            def conv_sgroup(ct, G):
                gk0 = ct * 4 + G * 2
                for gk in (gk0, gk0 + 1, gk0 + 2):
                    ensure_xg(gk)
                s1 = p_s1.tile([128, 32 * CCOL], BF16, name=f"s1_{ct}_{G}", tag="s1")
                for half in range(2):
                    g = G * 2 + half
                    t0 = ct * NQ + g * 16
                    xgv = xgs[gk0 + half][:].rearrange(
                        "p (q b jj) -> p q b jj", q=16, b=BS
                    )
                    for qi in range(16):
                        if qi % 2 == 0:
                            wst = p_wst.tile(
                                [128, 2 * 7 * 128], BF16,
                                name=f"ws_{ct}_{g}_{qi}", tag="wst",
                            )
                            nc.sync.dma_start(wst[:], d_wsb.ap()[(t0 + qi) // 2])
                        pc = ps_conv.tile(
                            [128, CCOL], F32, name=f"pc_{ct}_{g}_{qi}", tag="pc"
                        )
                        wbase = (qi % 2) * 896
                        for dj in range(7):
                            nc.tensor.matmul(
                                pc[:],
                                wst[:, wbase + dj * 128:wbase + (dj + 1) * 128],
                                xgv[:, qi, :, dj:dj + W],
                                start=(dj == 0),
                                stop=(dj == 6),
                            )
                        qo = half * 16 + qi
                        nc.vector.tensor_copy(s1[:, qo * CCOL:(qo + 1) * CCOL], pc[:])
                s1v = s1[:].rearrange("p (q b j) -> p q b j", q=32, b=BS)
                # scatter-writes on the 2nd HWDGE ring (ACT is idle in
                # phase A), b-major so image 0's Y read can fire during the
                # remaining drain at the phase A -> B transition
                for b in range(BS):
                    for c2 in range(2):
                        nc.scalar.dma_start(
                            d_T.ap()[ct, c2, b, G * 32:(G + 1) * 32].rearrange(
                                "q r j -> r q j"
                            ),
                            s1v[c2 * 64:c2 * 64 + H, :, b, :],
                        )

            def read_y(ct, G):
                # Y partition layout: c = 2q + c2, so each 64-partition read
                # depends only on super-group G's T writes (tiny launch wait)
                for b in range(BS):
                    nc.sync.dma_start(
                        Y[ct][G * 64:(G + 1) * 64, b * PIX:(b + 1) * PIX],
                        d_T.ap()[ct, :, b, G * 32:(G + 1) * 32].rearrange(
                            "c2 q r j -> q c2 (r j)"
                        ),
                    )

            # emit each Y read one super-group late: its T-write deps are then
            # already (nearly) complete, so the launch never blocks the ring
            sgs = [(0, 0), (0, 1), (1, 0), (1, 1)]
            for k, (ct, G) in enumerate(sgs):
                conv_sgroup(ct, G)
                if k >= 1:
                    read_y(*sgs[k - 1])
            read_y(*sgs[-1])

            # ---------------- phase B: LN stats + MLP ----------------
            mus = {}
            vrs = {}

            def stats_chunk(i, rb):
                sl = slice((i * NRB + rb) * CW, (i * NRB + rb + 1) * CW)
                msl = slice(rb * CW, (rb + 1) * CW)
                if rb == 0:
                    mus[i] = p_mu.tile([128, PIX], F32, name=f"mu_{i}", tag="mu")
                    mus["va", i] = p_va.tile([128, PIX], F32, name=f"va_{i}", tag="va")
                mu = mus[i]
                va = mus["va", i]
                y2 = p_y2.tile([128, 2 * CW], BF16, name=f"y2_{i}_{rb}", tag="y2")
                nc.vector.tensor_tensor(y2[:, 0:CW], Y[0][:, sl], Y[0][:, sl], op=ALU.mult)
                nc.vector.tensor_tensor(y2[:, CW:], Y[1][:, sl], Y[1][:, sl], op=ALU.mult)
                psy = ps_sy.tile([128, CW], F32, name=f"psy_{i}_{rb}", tag="psy")
                nc.tensor.matmul(psy[:], ones128[:], Y[0][:, sl], start=True, stop=False)
                nc.tensor.matmul(psy[:], ones128[:], Y[1][:, sl], start=False, stop=True)
                psy2 = ps_sy2.tile([128, CW], F32, name=f"psy2_{i}_{rb}", tag="psy2")
                nc.tensor.matmul(psy2[:], ones128[:], y2[:, 0:CW], start=True, stop=False)
                nc.tensor.matmul(psy2[:], ones128[:], y2[:, CW:], start=False, stop=True)
                nc.vector.tensor_scalar(mu[:, msl], psy[:], 1.0 / DIM, None, op0=ALU.mult)
                nc.vector.tensor_tensor(va[:, msl], mu[:, msl], mu[:, msl], op=ALU.mult)
                nc.vector.scalar_tensor_tensor(
                    va[:, msl], psy2[:], 1.0 / DIM, va[:, msl],
                    op0=ALU.mult, op1=ALU.subtract,
                )
            def stats_fin(i):
                # r = 1/sqrt(var + eps), batched per image (one ACT table use)
                vr = p_vr.tile([128, PIX], BF16, name=f"vr_{i}", tag="vr")
                vrs[i] = vr
                nc.scalar.activation(
                    vr[:], mus["va", i][:], AF.Abs_reciprocal_sqrt, bias=cst[:, 10:11]
                )

            def mlp_chunk(i, rb):
                sl = slice((i * NRB + rb) * CW, (i * NRB + rb + 1) * CW)
                msl = slice(rb * CW, (rb + 1) * CW)
                mu = mus[i]
                vr = vrs[i]
                yh = p_yh.tile([128, 2 * CW], BF16, name=f"yh_{i}_{rb}", tag="yh")
                for ct2 in range(2):
                    ysl = yh[:, ct2 * CW:(ct2 + 1) * CW]
                    nc.vector.scalar_tensor_tensor(
                        ysl, mu[:, msl], -1.0, Y[ct2][:, sl], op0=ALU.mult, op1=ALU.add
                    )
                    nc.gpsimd.tensor_tensor(ysl, ysl, vr[:, msl], op=ALU.mult)
                hb = p_h.tile([128, 8 * CW], BF16, name=f"hb_{i}_{rb}", tag="hb")
                for f in range(8):
                    ph = ps_h.tile([128, CW], F32, name=f"ph_{i}_{rb}_{f}", tag="ph")
                    nc.tensor.matmul(
                        ph[:], w1sb[:, f * 128:(f + 1) * 128], yh[:, 0:CW],
                        start=True, stop=False,
                    )
                    nc.tensor.matmul(
                        ph[:], w1sb[:, 1024 + f * 128:1024 + (f + 1) * 128],
                        yh[:, CW:], start=False, stop=True,
                    )
                    nc.scalar.activation(
                        hb[:, f * CW:(f + 1) * CW], ph[:], AF.Gelu,
                        bias=cst[:, f:f + 1],
                    )
                for ct2 in range(2):
                    po = ps_o.tile([128, CW], F32, name=f"po_{i}_{rb}_{ct2}", tag="po")
                    for f in range(8):
                        nc.tensor.matmul(
                            po[:], w2sb[:, f * 256 + ct2 * 128:f * 256 + (ct2 + 1) * 128],
                            hb[:, f * CW:(f + 1) * CW],
                            start=(f == 0), stop=(f == 7),
                        )
                    oc = p_oc.tile([128, CW], F32, name=f"oc_{i}_{rb}_{ct2}", tag="oc")
                    nc.vector.tensor_scalar(
                        oc[:], po[:], cst[:, 8 + ct2:9 + ct2], None, op0=ALU.add
                    )
                    nc.sync.dma_start(
                        d_out.ap()[i, ct2, :, rb * CW:(rb + 1) * CW], oc[:]
                    )

            def phase_b():
                for rb in range(NRB):
                    stats_chunk(0, rb)
                stats_fin(0)
                # front-load image i+1's stats into the first half of image
                # i's MLP so the rsqrt hides under the remaining MLP matmuls
                stats_slot = {0: [0, 1], 1: [2, 3], 2: [4, 5], 3: [6]}
                for i in range(BS):
                    for rb in range(NRB):
                        mlp_chunk(i, rb)
                        if i + 1 < BS:
                            for sc in stats_slot.get(rb, []):
                                stats_chunk(i + 1, sc)
                            if rb == 3:
                                stats_fin(i + 1)

            phase_b()

    nc.compile()
    return nc


def _host_prep(x, conv_w, conv_b, ln_g, ln_b, w1, b1, w2, b2):
    """Returns (shared static arrays dict, per-core xcv list)."""
    import concourse.mybir as mybir

    f32 = np.float32
    bf16 = np.dtype(mybir.dt.np(mybir.dt.bfloat16))
    x = np.asarray(x, f32)
    conv_w = np.asarray(conv_w, f32)
    conv_b = np.asarray(conv_b, f32)
    ln_g = np.asarray(ln_g, f32)
    ln_b = np.asarray(ln_b, f32)
    w1 = np.asarray(w1, f32)
    b1 = np.asarray(b1, f32)
    w2 = np.asarray(w2, f32)
    b2 = np.asarray(b2, f32)

    # fold LN affine into w1/b1
    w1g = (ln_g[:, None] * w1).astype(f32)                  # [256, 1024]
    b1e = (ln_b @ w1 + b1).astype(f32)                      # [1024]

    # Y partition layout is c = 2q + c2 (so per-super-group reads are
    # contiguous); permute w1's contraction rows to match
    qv = np.arange(NQ)
    perm = np.empty(128, np.int64)
    perm[2 * qv] = qv
    perm[2 * qv + 1] = 64 + qv
    w1sb = np.ascontiguousarray(
        w1g.reshape(2, 128, 8, 128)[:, perm].transpose(1, 0, 2, 3).reshape(128, 2048)
    ).astype(bf16)
    w2sb = np.ascontiguousarray(
        w2.reshape(8, 128, 2, 128).transpose(1, 0, 2, 3).reshape(128, 2048)
    ).astype(bf16)
    ones128 = np.ones((128, 128), bf16)

    cst = np.zeros((128, 11), f32)
    cst[:, 0:8] = b1e.reshape(8, 128).T
    cst[:, 8] = b2[:128]
    cst[:, 9] = b2[128:]
    cst[:, 10] = EPS

    # banded stationaries: wsb[ct, q, k, dj, m]
    wsb = np.zeros((2, NQ, 128, 7, 128), f32)
    w4 = conv_w.reshape(2, 2, NQ, 7, 7)                     # [ct, c2, q, di, dj]
    cb4 = conv_b.reshape(2, 2, NQ)
    ro = np.arange(H)
    for c2 in range(2):
        for di in range(7):
            for dj in range(7):
                wsb[:, :, c2 * 64 + ro + di, dj, c2 * 64 + ro] = \
                    w4[:, c2, :, di, dj][:, :, None]
        wsb[:, :, 62, 0, c2 * 64:c2 * 64 + H] = cb4[:, c2, :, None]
    # regroup: pairs of 2 q-tiles, per-partition contiguous
    wsb = np.ascontiguousarray(
        wsb.reshape(64, 2, 128, 7 * 128).transpose(0, 2, 1, 3).reshape(64, 128, 1792)
    ).astype(bf16)

    # padded input grids, conv layout
    xg = np.zeros((B, DIM, GR, GJ), f32)
    xg[:, :, 3:59, 3:59] = x
    xcvs = []
    for c in range(NCORES):
        xc = xg[c * BS:(c + 1) * BS].reshape(BS, 2, 2, NQ, GR, GJ)
        xcv = np.zeros((2, NQ, 128, BS, GJ), f32)
        xct = xc.transpose(1, 3, 2, 4, 0, 5)                # [ct, q, c2, r, b, jj]
        for c2 in range(2):
            xcv[:, :, c2 * 64:c2 * 64 + GR] = xct[:, :, c2]
        xcv[:, :, 62] = 1.0
        xcvs.append(
            np.ascontiguousarray(
                xcv.reshape(8, 16, 128, BS * GJ)
                .transpose(0, 2, 1, 3)
                .reshape(8, 128, 16 * BS * GJ)
            ).astype(bf16)
        )

    static = dict(w1sb=w1sb, w2sb=w2sb, wsb=wsb, ones128=ones128, cstf=cst)
    return static, xcvs


def kernel(**inputs) -> np.ndarray:
    from concourse import bass_utils

    if "nc" not in _CACHE:
        _CACHE["nc"] = _build_program()
    nc = _CACHE["nc"]

    static, xcvs = _host_prep(**inputs)
    in_maps = [dict(static, xcv=xcvs[c]) for c in range(NCORES)]
    res = bass_utils.run_bass_kernel_spmd(nc, in_maps, core_ids=list(range(NCORES)))

    out = np.empty((B, DIM, H, W), np.float32)
    for c in range(NCORES):
        yo = res.results[c]["yout"].reshape(BS, NCT, 128, H, W)
        for b in range(BS):
            out[c * BS + b, :128] = yo[b, 0]
            out[c * BS + b, 128:] = yo[b, 1]
    return out
